# revision 21
# baseline (speedup 1.0000x reference)
"""Sparse-attention Trainium2 kernel (nn_Attention_81398220193933).

Strategy (tensor-parallel over heads, 2 heads per NeuronCore):
  - Host pre-lays-out per-core tensors:
      qT  [B, 128, S]  bf16 : rows 0:64 = headA Q^T / sqrt(dh), rows 64:128 = headB
      kT  [B, 128, S]  bf16 : same for K^T
      vE  [B, 128, 8, 130] bf16 : per k-tile t, partition p = key position t*128+p,
           cols [0:64]=V_A*emb, [64]=emb, [65:129]=V_B*emb, [129]=emb
           where emb[b,k] = exp(bias[k]) * (k < seq_len[b]) (all-valid if seq_len==0).
    Folding the additive key bias + mask multiplicatively into V makes the
    softmax mask/bias free on-device and lets fully-masked k-tiles be skipped.
  - Device, per batch b and key-tile t (Kb = ceil(seq_len/128) tiles):
      scores^T [k=128, q=1024] = K_tile^T.T @ Q^T  (per head, 64-row lhsT)
      W^T = exp(scores^T), bf16, split across TWO engines:
        * ScalarE: exact table exp (PSUM f32 -> SBUF bf16), ~1.11us/tile
        * DVE: 1-op Schraudolph fast-exp -- tensor_scalar(mult 2^23/ln2/2^16,
          add (127*2^23-shift)/2^16) with int16 output; the f32->int16
          round-to-nearest convert yields exactly the high 16 bits of the
          Schraudolph int32, i.e. a valid bf16 W tile (~3% max elem error,
          washes to ~1e-2 output error after softmax normalization).
      Tiles are assigned greedily to whichever engine's queue is shorter,
      so exp throughput is the SUM of both engines (~2x ScalarE alone).
      out[q,65] += W^T_chunk.T @ V_tile accumulated over t into a single
      2-bank PSUM acc tile [128, 2, 4, 128] (8 q-chunk regions of 65 cols);
      column 64 accumulates the softmax denominator via the emb column.
    Per-head epilogue: ONE DVE reciprocal over the 8 denominators + ONE
    tensor_tensor multiply [128,2,4,64] -> ot, then DMA to HBM.
  - PSUM: 3 x [128,1024] f32 score bufs (6 banks) + 1 x acc (2 banks) = 8.
  - AV regions of batch b drip between QK steps of batch b+1 (PE stays
    ~90% busy so HAM holds 2.4 GHz without junk matmuls; a short warmup
    burst covers the initial DMA window).
  - Softmax max-subtraction unnecessary: logits are O(+-9) and masked keys
    contribute exactly zero through emb (matches the jax reference).
"""

import numpy as np
import ml_dtypes

import concourse.bass as bass
import concourse.mybir as mybir
import concourse.tile as tile
from concourse import bacc
from concourse.bass_utils import run_bass_kernel_spmd

B = 8
S = 1024
UNITS = 1024
H = 16
DH = 64
N_CORES = 8
KT = S // 128  # max key tiles per batch

BF16 = mybir.dt.bfloat16
F32 = mybir.dt.float32
I16 = mybir.dt.int16

# Schraudolph fast-exp constants, scaled so the f32->int16 convert emits
# the high 16 bits (bf16) directly.
EXP_SHIFT = 366393.0
A16 = float(2.0**23 / np.log(2.0) / 65536.0)
B16 = float((127.0 * 2.0**23 - EXP_SHIFT) / 65536.0)

# engine-balance cost model (ns, measured on HW) for the greedy assignment
COST_S = 1105.0   # ScalarE exp [128,1024]
COST_D = 1250.0   # DVE fast-exp [128,1024]
COST_EPI = 710.0   # DVE per-half epilogue (recip + tensor_tensor)


def _build_nc(kbs):
    """Build the SPMD Bass program. kbs: per-batch number of 128-key tiles."""
    nc = bacc.Bacc("TRN2", target_bir_lowering=False, debug=False,
                   num_devices=N_CORES)
    qT = nc.dram_tensor("qt", [B, 128, S], BF16, kind="ExternalInput").ap()
    kT = nc.dram_tensor("kt", [B, 128, S], BF16, kind="ExternalInput").ap()
    vE = nc.dram_tensor("vt", [B, 128, KT, 130], BF16, kind="ExternalInput").ap()
    o = nc.dram_tensor("o", [B, S, 128], F32, kind="ExternalOutput").ap()

    with tile.TileContext(nc) as tc:
        with (
            tc.tile_pool(name="qk", bufs=2) as qk_pool,
            tc.tile_pool(name="v", bufs=2) as v_pool,
            tc.tile_pool(name="w", bufs=40) as w_pool,
            tc.tile_pool(name="ot", bufs=3) as o_pool,
            tc.tile_pool(name="rc", bufs=4) as r_pool,
            tc.tile_pool(name="sc", bufs=3, space="PSUM") as sc_pool,
            tc.tile_pool(name="acc", bufs=2, space="PSUM") as acc_pool,
        ):
            # --- exp-table preload + HAM warmup (overlap the input DMAs) ---
            wexp = qk_pool.tile([1, 8], F32, tag="wexp", name="wexp", bufs=1)
            nc.vector.memset(wexp[:], 0.0)
            nc.scalar.activation(wexp[:], wexp[:],
                                 mybir.ActivationFunctionType.Exp)
            wu = qk_pool.tile([128, 640], BF16, tag="wu", name="wu")
            nc.vector.memset(wu[:], 0.0)

            # Load every batch's inputs up front (fits easily in SBUF).
            # First batch small (warms up on real work at low cost), then
            # largest-first, smallest last (short tail after the final exp).
            srt = sorted(range(B), key=lambda i: -kbs[i])
            order = [srt[-2]] + srt[:-2] + [srt[-1]]
            qts, kts, vts = {}, {}, {}
            for b in order:
                qts[b] = qk_pool.tile([128, S], BF16, tag=f"qt{b}",
                                      name=f"qt{b}", bufs=1)
                kts[b] = qk_pool.tile([128, S], BF16, tag=f"kt{b}",
                                      name=f"kt{b}", bufs=1)
                vts[b] = v_pool.tile([128, kbs[b], 130], BF16, tag=f"vt{b}",
                                     name=f"vt{b}", bufs=1)
            # DMA order: first two batches' Q/K, first batch's V (its AV
            # drip starts one batch in), then the rest, remaining V last.
            for b in order[:2]:
                nc.sync.dma_start(out=qts[b][:], in_=qT[b])
                nc.sync.dma_start(out=kts[b][:], in_=kT[b])
            nc.sync.dma_start(out=vts[order[0]][:],
                              in_=vE[order[0], :, :kbs[order[0]], :])
            for b in order[2:]:
                nc.sync.dma_start(out=qts[b][:], in_=qT[b])
                nc.sync.dma_start(out=kts[b][:], in_=kT[b])
            for b in order[1:]:
                nc.sync.dma_start(out=vts[b][:], in_=vE[b, :, :kbs[b], :])

            # HAM warmup burst into the first sc slots (freed for real QK
            # by pool rotation): ~8.5us of sustained PE activity at the cold
            # 1.2 GHz clock -- two full free-running HAM windows, so the
            # 2.4 GHz promote fires regardless of window phase (a shorter
            # burst left some cores throttled for 40+us).
            for i in range(17):
                if i % 9 == 0:
                    wsc = sc_pool.tile([128, S], F32, tag="sc", name="scwu")
                nc.tensor.matmul(wsc[:, (i % 2) * 512:(i % 2) * 512 + 512],
                                 lhsT=wu[:, 0:128], rhs=wu[:, 128:640],
                                 start=True, stop=True, skip_group_check=True)

            # --- greedy engine balance state ---
            eng_ns = {"s": 1283.0, "d": 0.0}  # ScalarE starts with table load

            def pick_engine(force=None):
                if force is None:
                    force = "s" if (eng_ns["s"] + COST_S
                                    <= eng_ns["d"] + COST_D) else "d"
                eng_ns[force] += COST_S if force == "s" else COST_D
                return force

            # --- AV drip machinery ---
            # Per head, the 8 q-chunk regions split into two 1-bank PSUM
            # accumulator halves (4 regions each). Each half has its own
            # epilogue, so a half is freed while the other still
            # accumulates -- the next consumer of the PSUM slot is ~5 drip
            # slots behind and never blocks the in-order PE queue.
            avq = []  # items: ("av", rec, h, j) | ("epi", rec, h, half)

            def emit_item(item):
                kind, rec, h, j = item
                b, kb, vt, wts = rec["b"], rec["kb"], rec["vt"], rec["wts"]
                if kind == "av":
                    if j % 4 == 0:
                        rec["acc"][(h, j // 4)] = acc_pool.tile(
                            [128, 4, 128], F32, tag="acc",
                            name=f"acc{b}_{h}_{j // 4}")
                    grp = rec["acc"][(h, j // 4)]
                    for t in range(kb):
                        nc.tensor.matmul(
                            grp[:, j % 4, 0:65],
                            lhsT=wts[h][t][:, j * 128:(j + 1) * 128],
                            rhs=vt[:, t, h * 65:h * 65 + 65],
                            start=(t == 0), stop=(t == kb - 1),
                        )
                else:  # epilogue for head h, half j
                    grp = rec["acc"][(h, j)]
                    rc = r_pool.tile([128, 4, 1], F32, tag="rc", name="rc")
                    nc.vector.reciprocal(rc[:], grp[:, :, 64:65])
                    rc_b = bass.AP(tensor=rc.tensor, offset=rc.offset,
                                   ap=[rc.ap[0], rc.ap[1], [0, 64]])
                    ot = o_pool.tile([128, 4, 64], F32, tag="ot", name="ot")
                    nc.vector.tensor_tensor(
                        ot[:], grp[:, :, 0:64], rc_b, mybir.AluOpType.mult)
                    ov = o[b].rearrange("(g j p) c -> p g j c", p=128, j=4)
                    nc.sync.dma_start(
                        out=ov[:, j, :, h * 64:(h + 1) * 64], in_=ot[:])

            def _head_items(rec, h):
                # charge the head's two epilogues to the DVE queue now so
                # the exp greedy sees the load before it lands
                eng_ns["d"] += 2 * COST_EPI
                return ([("av", rec, h, j) for j in range(4)]
                        + [("epi", rec, h, 0)]
                        + [("av", rec, h, j) for j in range(4, 8)]
                        + [("epi", rec, h, 1)])

            total_half = sum(2 * kbs[b] for b in order)
            half_no = 0

            def drip():
                """Emit queued AV/epilogue items at a steady 1-3 per
                half-step (~160 items over ~144 half-steps) so the PE
                always has score-independent work to fill micro-idles
                (keeps the HAM clock gate at 2.4 GHz). While the queue is
                empty (first batches), emit a small junk matmul instead --
                an early PE hole demotes the clock and the re-promote needs
                ~3us of continuous execution."""
                # Unconditional junk LDWEIGHTS (~50ns, reads only the
                # constant wu tile -- no PSUM write, no dependencies, so it
                # really fills PE idle): keeps every free-running HAM window
                # near-100% PE-busy so the clock gate rarely demotes, and a
                # saturated MID window re-promotes at the next ~3.4us
                # boundary instead of drifting for 10-20us. The next real
                # matmul reloads its own weights, so clobbering is harmless.
                nc.tensor.ldweights(wu[:, 0:128])
                if not avq:
                    for _ in range(3):
                        nc.tensor.ldweights(wu[:, 0:128])
                    return
                rem = max(1, total_half - half_no - 4)
                rate = -(-len(avq) // rem)
                for _ in range(min(max(rate, 1), 2)):
                    if avq:
                        emit_item(avq.pop(0))

            for bi, b in enumerate(order):
                kb = kbs[b]
                qt, kt, vt = qts[b], kts[b], vts[b]
                wts = [[], []]
                rec = {"b": b, "kb": kb, "wts": wts, "vt": vt, "acc": {}}
                last = bi == len(order) - 1
                # Last batch: all of head A's tiles first (forced ScalarE),
                # then head B's (forced DVE), so A's AV+epilogue overlap B's
                # exp phase instead of extending the kernel tail.
                if last:
                    step_list = [(t, h) for h in range(2) for t in range(kb)]
                else:
                    step_list = [(t, h) for t in range(kb) for h in range(2)]
                for t, h in step_list:
                    half_no += 1
                    base = 64 * h
                    sc = sc_pool.tile([128, S], F32, tag="sc", name="sc")
                    for qc in range(2):
                        nc.tensor.matmul(
                            sc[:, qc * 512:(qc + 1) * 512],
                            lhsT=kt[base:base + 64, t * 128:(t + 1) * 128],
                            rhs=qt[base:base + 64, qc * 512:(qc + 1) * 512],
                            start=True, stop=True,
                        )
                    wt = w_pool.tile([128, S], BF16, tag="w",
                                     name=f"w{b}_{t}_{h}")
                    eng = pick_engine(("d" if h == 0 else "s") if last else None)
                    if eng == "s":
                        nc.scalar.activation(wt[:], sc[:],
                                             mybir.ActivationFunctionType.Exp)
                    else:
                        nc.vector.tensor_scalar(
                            wt[:].bitcast(I16), sc[:], A16, B16,
                            mybir.AluOpType.mult, mybir.AluOpType.add)
                    wts[h].append(wt)
                    if last and h == 0 and t == kb - 1:
                        avq.extend(_head_items(rec, 0))
                    # drip AFTER this half-step's QK+exp: AV work fills the
                    # PE behind them without delaying the exp stream
                    drip()
                # batch finished: queue AV regions + epilogues
                if last:
                    avq.extend(_head_items(rec, 1))
                else:
                    for h in range(2):
                        avq.extend(_head_items(rec, h))

            while avq:
                emit_item(avq.pop(0))
                for _ in range(3):
                    nc.tensor.ldweights(wu[:, 0:128])
    nc.compile()
    return nc


_NC_CACHE = {}


def _get_nc(kbs):
    key = tuple(kbs)
    if key not in _NC_CACHE:
        _NC_CACHE[key] = _build_nc(key)
    return _NC_CACHE[key]


def kernel(memory, query, b, seq_len):
    memory = np.asarray(memory)
    query = np.asarray(query)
    bias = np.asarray(b, dtype=np.float32)
    seq_len = np.asarray(seq_len).reshape(-1).astype(np.int64)

    sl = seq_len.copy()
    kbs = [int(min(KT, max(1, -(-int(s) // 128)))) if s > 0 else KT for s in sl]

    # emb[b, k] = exp(bias[k]) * valid; fully-masked batch -> plain softmax
    pos = np.arange(S)[None, :]
    valid = (pos < sl[:, None]) | (sl[:, None] == 0)
    emb = np.exp(bias)[None, :] * valid.astype(np.float32)  # [B, S]

    qh = (query.astype(np.float32) * (DH ** -0.5)).reshape(B, S, H, DH)
    kh = memory[:, :, :UNITS].astype(np.float32).reshape(B, S, H, DH)
    vh = memory[:, :, UNITS:].astype(np.float32).reshape(B, S, H, DH)
    vh = vh * emb[:, :, None, None]  # [B, S, H, DH] value rows pre-masked

    bf = ml_dtypes.bfloat16
    # [B, S, H, DH] -> [B, H, DH, S] transposed layouts
    qTfull = np.ascontiguousarray(qh.transpose(0, 2, 3, 1)).astype(bf)
    kTfull = np.ascontiguousarray(kh.transpose(0, 2, 3, 1)).astype(bf)
    # [B, S, H, DH] -> [B, (t p), H, DH] -> [B, 128, KT, H, DH]
    vtiles = np.ascontiguousarray(
        vh.reshape(B, KT, 128, H, DH).transpose(0, 2, 1, 3, 4)).astype(bf)
    embt = np.ascontiguousarray(
        emb.reshape(B, KT, 128).transpose(0, 2, 1)).astype(bf)  # [B, 128, KT]

    in_maps = []
    for c in range(N_CORES):
        hA, hB = 2 * c, 2 * c + 1
        qT = np.concatenate([qTfull[:, hA], qTfull[:, hB]], axis=1)  # [B,128,S]
        kT = np.concatenate([kTfull[:, hA], kTfull[:, hB]], axis=1)
        vEc = np.empty((B, 128, KT, 130), dtype=bf)
        vEc[..., 0:64] = vtiles[:, :, :, hA, :]
        vEc[..., 64] = embt
        vEc[..., 65:129] = vtiles[:, :, :, hB, :]
        vEc[..., 129] = embt
        in_maps.append({
            "qt": np.ascontiguousarray(qT),
            "kt": np.ascontiguousarray(kT),
            "vt": np.ascontiguousarray(vEc),
        })

    nc = _get_nc(kbs)
    res = run_bass_kernel_spmd(nc, in_maps, core_ids=list(range(N_CORES)))

    out = np.empty((B, S, UNITS), dtype=np.float32)
    for c in range(N_CORES):
        out[:, :, 128 * c:128 * (c + 1)] = res.results[c]["o"]
    return out


# revision 22
# speedup vs baseline: 1.5076x; 1.5076x over previous
"""Sparse-attention Trainium2 kernel (nn_Attention_81398220193933).

Strategy (tensor-parallel over heads, 2 heads per NeuronCore):
  - Host pre-lays-out per-core tensors:
      qT  [B, 128, S]  bf16 : rows 0:64 = headA Q^T / sqrt(dh), rows 64:128 = headB
      kT  [B, 128, S]  bf16 : same for K^T
      vE  [B, 128, 8, 130] bf16 : per k-tile t, partition p = key position t*128+p,
           cols [0:64]=V_A*emb, [64]=emb, [65:129]=V_B*emb, [129]=emb
           where emb[b,k] = exp(bias[k]) * (k < seq_len[b]) (all-valid if seq_len==0).
    Folding the additive key bias + mask multiplicatively into V makes the
    softmax mask/bias free on-device and lets fully-masked k-tiles be skipped.
  - Device, per batch b and key-tile t (Kb = ceil(seq_len/128) tiles):
      scores^T [k=128, q=1024] = K_tile^T.T @ Q^T  (per head, 64-row lhsT)
      W^T = exp(scores^T), bf16, split across TWO engines:
        * ScalarE: exact table exp (PSUM f32 -> SBUF bf16), ~1.11us/tile
        * DVE: 1-op Schraudolph fast-exp -- tensor_scalar(mult 2^23/ln2/2^16,
          add (127*2^23-shift)/2^16) with int16 output; the f32->int16
          round-to-nearest convert yields exactly the high 16 bits of the
          Schraudolph int32, i.e. a valid bf16 W tile (~3% max elem error,
          washes to ~1e-2 output error after softmax normalization).
      Tiles are assigned greedily to whichever engine's queue is shorter,
      so exp throughput is the SUM of both engines (~2x ScalarE alone).
      out[q,65] += W^T_chunk.T @ V_tile accumulated over t into a single
      2-bank PSUM acc tile [128, 2, 4, 128] (8 q-chunk regions of 65 cols);
      column 64 accumulates the softmax denominator via the emb column.
    Per-head epilogue: ONE DVE reciprocal over the 8 denominators + ONE
    tensor_tensor multiply [128,2,4,64] -> ot, then DMA to HBM.
  - PSUM: 3 x [128,1024] f32 score bufs (6 banks) + 1 x acc (2 banks) = 8.
  - AV regions of batch b drip between QK steps of batch b+1 (PE stays
    ~90% busy so HAM holds 2.4 GHz without junk matmuls; a short warmup
    burst covers the initial DMA window).
  - Softmax max-subtraction unnecessary: logits are O(+-9) and masked keys
    contribute exactly zero through emb (matches the jax reference).
"""

import numpy as np
import ml_dtypes

import concourse.bass as bass
import concourse.mybir as mybir
import concourse.tile as tile
from concourse import bacc
from concourse.bass_utils import run_bass_kernel_spmd

B = 8
S = 1024
UNITS = 1024
H = 16
DH = 64
N_CORES = 8
KT = S // 128  # max key tiles per batch

BF16 = mybir.dt.bfloat16
F32 = mybir.dt.float32
I16 = mybir.dt.int16

# Schraudolph fast-exp constants, scaled so the f32->int16 convert emits
# the high 16 bits (bf16) directly.
EXP_SHIFT = 366393.0
A16 = float(2.0**23 / np.log(2.0) / 65536.0)
B16 = float((127.0 * 2.0**23 - EXP_SHIFT) / 65536.0)

# engine-balance cost model (ns, measured on HW) for the greedy assignment
COST_S = 1105.0   # ScalarE exp [128,1024]
COST_D = 1250.0   # DVE fast-exp [128,1024]
COST_EPI = 710.0   # DVE per-half epilogue (recip + tensor_tensor)


def _build_nc(kbs):
    """Build the SPMD Bass program. kbs: per-batch number of 128-key tiles."""
    nc = bacc.Bacc("TRN2", target_bir_lowering=False, debug=False,
                   num_devices=N_CORES)
    qT = nc.dram_tensor("qt", [B, 128, S], BF16, kind="ExternalInput").ap()
    kT = nc.dram_tensor("kt", [B, 128, S], BF16, kind="ExternalInput").ap()
    vE = nc.dram_tensor("vt", [B, 128, KT, 130], BF16, kind="ExternalInput").ap()
    o = nc.dram_tensor("o", [B, S, 128], F32, kind="ExternalOutput").ap()

    with tile.TileContext(nc) as tc:
        with (
            tc.tile_pool(name="qk", bufs=2) as qk_pool,
            tc.tile_pool(name="v", bufs=2) as v_pool,
            tc.tile_pool(name="w", bufs=40) as w_pool,
            tc.tile_pool(name="ot", bufs=3) as o_pool,
            tc.tile_pool(name="rc", bufs=4) as r_pool,
            tc.tile_pool(name="sc", bufs=3, space="PSUM") as sc_pool,
            tc.tile_pool(name="acc", bufs=2, space="PSUM") as acc_pool,
        ):
            # --- exp-table preload + HAM warmup (overlap the input DMAs) ---
            wexp = qk_pool.tile([1, 8], F32, tag="wexp", name="wexp", bufs=1)
            nc.vector.memset(wexp[:], 0.0)
            nc.scalar.activation(wexp[:], wexp[:],
                                 mybir.ActivationFunctionType.Exp)
            wu = qk_pool.tile([128, 640], BF16, tag="wu", name="wu")
            nc.vector.memset(wu[:], 0.0)

            # Load every batch's inputs up front (fits easily in SBUF).
            # First batch small (warms up on real work at low cost), then
            # largest-first, smallest last (short tail after the final exp).
            srt = sorted(range(B), key=lambda i: -kbs[i])
            order = [srt[-2]] + srt[:-2] + [srt[-1]]
            qts, kts, vts = {}, {}, {}
            for b in order:
                qts[b] = qk_pool.tile([128, S], BF16, tag=f"qt{b}",
                                      name=f"qt{b}", bufs=1)
                kts[b] = qk_pool.tile([128, S], BF16, tag=f"kt{b}",
                                      name=f"kt{b}", bufs=1)
                vts[b] = v_pool.tile([128, kbs[b], 130], BF16, tag=f"vt{b}",
                                     name=f"vt{b}", bufs=1)
            # DMA order: first two batches' Q/K, first batch's V (its AV
            # drip starts one batch in), then the rest, remaining V last.
            for b in order[:2]:
                nc.sync.dma_start(out=qts[b][:], in_=qT[b])
                nc.sync.dma_start(out=kts[b][:], in_=kT[b])
            nc.sync.dma_start(out=vts[order[0]][:],
                              in_=vE[order[0], :, :kbs[order[0]], :])
            for b in order[2:]:
                nc.sync.dma_start(out=qts[b][:], in_=qT[b])
                nc.sync.dma_start(out=kts[b][:], in_=kT[b])
            for b in order[1:]:
                nc.sync.dma_start(out=vts[b][:], in_=vE[b, :, :kbs[b], :])

            # HAM warmup burst into the first sc slots (freed for real QK
            # by pool rotation): ~8.5us of sustained PE activity at the cold
            # 1.2 GHz clock -- two full free-running HAM windows, so the
            # 2.4 GHz promote fires regardless of window phase (a shorter
            # burst left some cores throttled for 40+us).
            for i in range(17):
                if i % 9 == 0:
                    wsc = sc_pool.tile([128, S], F32, tag="sc", name="scwu")
                nc.tensor.matmul(wsc[:, (i % 2) * 512:(i % 2) * 512 + 512],
                                 lhsT=wu[:, 0:128], rhs=wu[:, 128:640],
                                 start=True, stop=True, skip_group_check=True)

            # --- greedy engine balance state ---
            eng_ns = {"s": 1283.0, "d": 0.0}  # ScalarE starts with table load

            def pick_engine(force=None):
                if force is None:
                    force = "s" if (eng_ns["s"] + COST_S
                                    <= eng_ns["d"] + COST_D) else "d"
                eng_ns[force] += COST_S if force == "s" else COST_D
                return force

            # --- AV drip machinery ---
            # Per head, the 8 q-chunk regions split into two 1-bank PSUM
            # accumulator halves (4 regions each). Each half has its own
            # epilogue, so a half is freed while the other still
            # accumulates -- the next consumer of the PSUM slot is ~5 drip
            # slots behind and never blocks the in-order PE queue.
            avq = []  # items: ("av", rec, h, j) | ("epi", rec, h, half)

            def emit_item(item):
                kind, rec, h, j = item
                b, kb, vt, wts = rec["b"], rec["kb"], rec["vt"], rec["wts"]
                if kind == "av":
                    if j % 4 == 0:
                        rec["acc"][(h, j // 4)] = acc_pool.tile(
                            [128, 4, 128], F32, tag="acc",
                            name=f"acc{b}_{h}_{j // 4}")
                    grp = rec["acc"][(h, j // 4)]
                    for t in range(kb):
                        nc.tensor.matmul(
                            grp[:, j % 4, 0:65],
                            lhsT=wts[h][t][:, j * 128:(j + 1) * 128],
                            rhs=vt[:, t, h * 65:h * 65 + 65],
                            start=(t == 0), stop=(t == kb - 1),
                        )
                elif kind == "rcp":  # epilogue part 1: reciprocal
                    grp = rec["acc"][(h, j)]
                    rc = r_pool.tile([128, 4, 1], F32, tag="rc", name="rc")
                    nc.vector.reciprocal(rc[:], grp[:, :, 64:65])
                    rec[("rc", h, j)] = rc
                else:  # "tt" -- epilogue part 2: multiply + store
                    grp = rec["acc"][(h, j)]
                    rc = rec[("rc", h, j)]
                    rc_b = bass.AP(tensor=rc.tensor, offset=rc.offset,
                                   ap=[rc.ap[0], rc.ap[1], [0, 64]])
                    ot = o_pool.tile([128, 4, 64], F32, tag="ot", name="ot")
                    nc.vector.tensor_tensor(
                        ot[:], grp[:, :, 0:64], rc_b, mybir.AluOpType.mult)
                    ov = o[b].rearrange("(g j p) c -> p g j c", p=128, j=4)
                    nc.sync.dma_start(
                        out=ov[:, j, :, h * 64:(h + 1) * 64], in_=ot[:])
                    rec["tt_done"] = True

            def _head_items(rec, h):
                # charge the head's two epilogues to the DVE queue now so
                # the exp greedy sees the load before it lands
                eng_ns["d"] += 2 * COST_EPI
                return ([("av", rec, h, j) for j in range(4)]
                        + [("rcp", rec, h, 0), ("tt", rec, h, 0)]
                        + [("av", rec, h, j) for j in range(4, 8)]
                        + [("rcp", rec, h, 1), ("tt", rec, h, 1)])

            total_half = sum(2 * kbs[b] for b in order)
            half_no = 0
            epi_recent = [0]

            def drip():
                """Emit queued AV/epilogue items at a steady 1-3 per
                half-step (~160 items over ~144 half-steps) so the PE
                always has score-independent work to fill micro-idles
                (keeps the HAM clock gate at 2.4 GHz). While the queue is
                empty (first batches), emit a small junk matmul instead --
                an early PE hole demotes the clock and the re-promote needs
                ~3us of continuous execution."""
                if not avq:
                    return
                rem = max(1, total_half - half_no - 4)
                rate = -(-len(avq) // rem)
                for _ in range(min(max(rate, 1), 2)):
                    if avq:
                        item = avq.pop(0)
                        emit_item(item)
                        if item[0] in ("rcp", "tt"):
                            epi_recent[0] = 2

            for bi, b in enumerate(order):
                kb = kbs[b]
                qt, kt, vt = qts[b], kts[b], vts[b]
                wts = [[], []]
                rec = {"b": b, "kb": kb, "wts": wts, "vt": vt, "acc": {}}
                last = bi == len(order) - 1
                # Last batch: all of head A's tiles first (forced ScalarE),
                # then head B's (forced DVE), so A's AV+epilogue overlap B's
                # exp phase instead of extending the kernel tail.
                if last:
                    step_list = [(t, h) for h in range(2) for t in range(kb)]
                else:
                    step_list = [(t, h) for t in range(kb) for h in range(2)]
                for t, h in step_list:
                    half_no += 1
                    base = 64 * h
                    sc = sc_pool.tile([128, S], F32, tag="sc", name="sc")
                    for qc in range(2):
                        nc.tensor.matmul(
                            sc[:, qc * 512:(qc + 1) * 512],
                            lhsT=kt[base:base + 64, t * 128:(t + 1) * 128],
                            rhs=qt[base:base + 64, qc * 512:(qc + 1) * 512],
                            start=True, stop=True,
                        )
                    wt = w_pool.tile([128, S], BF16, tag="w",
                                     name=f"w{b}_{t}_{h}")
                    if last:
                        force = "d" if h == 0 else "s"
                    elif epi_recent[0] > 0:
                        epi_recent[0] -= 1
                        force = "s"
                    else:
                        force = None
                    eng = pick_engine(force)
                    if eng == "s":
                        nc.scalar.activation(wt[:], sc[:],
                                             mybir.ActivationFunctionType.Exp)
                    else:
                        nc.vector.tensor_scalar(
                            wt[:].bitcast(I16), sc[:], A16, B16,
                            mybir.AluOpType.mult, mybir.AluOpType.add)
                    wts[h].append(wt)
                    if last and h == 0 and t == kb - 1:
                        avq.extend(_head_items(rec, 0))
                    # drip AFTER this half-step's QK+exp: AV work fills the
                    # PE behind them without delaying the exp stream
                    drip()
                # batch finished: queue AV regions + epilogues
                if last:
                    avq.extend(_head_items(rec, 1))
                else:
                    for h in range(2):
                        avq.extend(_head_items(rec, h))

            while avq:
                emit_item(avq.pop(0))
    nc.compile()
    return nc


_NC_CACHE = {}


def _get_nc(kbs):
    key = tuple(kbs)
    if key not in _NC_CACHE:
        _NC_CACHE[key] = _build_nc(key)
    return _NC_CACHE[key]


def kernel(memory, query, b, seq_len):
    memory = np.asarray(memory)
    query = np.asarray(query)
    bias = np.asarray(b, dtype=np.float32)
    seq_len = np.asarray(seq_len).reshape(-1).astype(np.int64)

    sl = seq_len.copy()
    kbs = [int(min(KT, max(1, -(-int(s) // 128)))) if s > 0 else KT for s in sl]

    # emb[b, k] = exp(bias[k]) * valid; fully-masked batch -> plain softmax
    pos = np.arange(S)[None, :]
    valid = (pos < sl[:, None]) | (sl[:, None] == 0)
    emb = np.exp(bias)[None, :] * valid.astype(np.float32)  # [B, S]

    qh = (query.astype(np.float32) * (DH ** -0.5)).reshape(B, S, H, DH)
    kh = memory[:, :, :UNITS].astype(np.float32).reshape(B, S, H, DH)
    vh = memory[:, :, UNITS:].astype(np.float32).reshape(B, S, H, DH)
    vh = vh * emb[:, :, None, None]  # [B, S, H, DH] value rows pre-masked

    bf = ml_dtypes.bfloat16
    # [B, S, H, DH] -> [B, H, DH, S] transposed layouts
    qTfull = np.ascontiguousarray(qh.transpose(0, 2, 3, 1)).astype(bf)
    kTfull = np.ascontiguousarray(kh.transpose(0, 2, 3, 1)).astype(bf)
    # [B, S, H, DH] -> [B, (t p), H, DH] -> [B, 128, KT, H, DH]
    vtiles = np.ascontiguousarray(
        vh.reshape(B, KT, 128, H, DH).transpose(0, 2, 1, 3, 4)).astype(bf)
    embt = np.ascontiguousarray(
        emb.reshape(B, KT, 128).transpose(0, 2, 1)).astype(bf)  # [B, 128, KT]

    in_maps = []
    for c in range(N_CORES):
        hA, hB = 2 * c, 2 * c + 1
        qT = np.concatenate([qTfull[:, hA], qTfull[:, hB]], axis=1)  # [B,128,S]
        kT = np.concatenate([kTfull[:, hA], kTfull[:, hB]], axis=1)
        vEc = np.empty((B, 128, KT, 130), dtype=bf)
        vEc[..., 0:64] = vtiles[:, :, :, hA, :]
        vEc[..., 64] = embt
        vEc[..., 65:129] = vtiles[:, :, :, hB, :]
        vEc[..., 129] = embt
        in_maps.append({
            "qt": np.ascontiguousarray(qT),
            "kt": np.ascontiguousarray(kT),
            "vt": np.ascontiguousarray(vEc),
        })

    nc = _get_nc(kbs)
    res = run_bass_kernel_spmd(nc, in_maps, core_ids=list(range(N_CORES)))

    out = np.empty((B, S, UNITS), dtype=np.float32)
    for c in range(N_CORES):
        out[:, :, 128 * c:128 * (c + 1)] = res.results[c]["o"]
    return out
